# revision 1
# baseline (speedup 1.0000x reference)
"""GCN layer kernel for 8 trn2 NeuronCores (SPMD, single launch).

Math:  out = D^-1/2 (A+I) D^-1/2 X W^T + b
Identity: the dense layer commutes with the diagonal scalings:
    out = D^-1/2 (A+I) D^-1/2 (X W^T) + b
so U = X@W^T (tiny) is computed first, then one big matmul A_hat @ (dinv*U).

Distribution: row-shard A_hat = A+I across 8 cores (strip = 1024 rows).
The host supplies each core's strip TRANSPOSED and cast to bf16
(at_hat[k, i] = A_hat[row i_local, k]), which
  * halves HBM traffic (16.8MB/core, ~47us at 358GB/s roofline), and
  * puts the contraction dim k on partitions, so no on-device transposes.

Per core:
  phase 1 (overlapped): stream at_hat tiles; U = X@W^T on PE; degrees
      deg[i] = sum_k at_hat[k, i] via ones-vector matmuls (PSUM accum).
  AllGather (only collective): 1024 local degrees -> full 8192 degree.
  phase 2: dinv = deg^-1/2 (sqrt+recip+Newton); Y = dinv*U (bf16);
      Z^T[f, i] = sum_k Y[k, f] at_hat[k, i], accumulated over 64 k-tiles
      with Y tiles stationary (512-wide streams);
      epilogue: PE-transpose Z^T tiles, scale rows by local dinv, + bias.

A is read from HBM exactly once, in bf16.
"""

import numpy as np
import ml_dtypes

N = 8192          # nodes
F = 128           # in/out feature dim
NCORES = 8
SR = N // NCORES  # strip rows per core = 1024
P = 128           # partitions / tile edge
IT = SR // P      # 8 local row tiles
JT = N // P       # 64 contraction tiles
HC = 512          # phase-2 / degree stream chunk (one PSUM bank of fp32)

_CACHE = {}


def _build_nc():
    import concourse.mybir as mybir
    from concourse import bass
    from concourse.tile import TileContext

    f32 = mybir.dt.float32
    bf16 = mybir.dt.bfloat16
    AF = mybir.ActivationFunctionType

    nc = bass.Bass(num_devices=NCORES)

    At_d = nc.declare_dram_parameter("at_hat", [N, SR], bf16, False)  # (A+I)strip^T
    Xt = nc.declare_dram_parameter("xt_bf", [P, N], bf16, False)      # X^T, bf16
    Wt = nc.declare_dram_parameter("wt_bf", [P, F], bf16, False)      # W^T, bf16
    Bb = nc.declare_dram_parameter("b_bc", [P, F], f32, False)        # bias bcast
    Idn = nc.declare_dram_parameter("ident", [P, P], f32, False)
    Sel = nc.declare_dram_parameter("sel", [JT, IT], f32, False)      # local one-hot
    out = nc.declare_dram_parameter("out", [SR, F], f32, True)

    degL = nc.dram_tensor("deg_local", [IT, P], f32)
    degA = nc.dram_tensor("deg_all", [JT, P], f32, addr_space="Shared")

    with TileContext(nc) as tc:
        with tc.tile_pool(name="const", bufs=1) as constp, \
             tc.tile_pool(name="big", bufs=1) as bigp, \
             tc.tile_pool(name="small", bufs=1) as smallp, \
             tc.tile_pool(name="outs", bufs=3) as outp, \
             tc.tile_pool(name="pdeg", bufs=1, space="PSUM") as pdeg, \
             tc.tile_pool(name="pu", bufs=2, space="PSUM") as pu, \
             tc.tile_pool(name="pzt", bufs=2, space="PSUM") as pzt, \
             tc.tile_pool(name="ptr", bufs=2, space="PSUM") as ptr:

            # ---- constants / small inputs ----
            ident = constp.tile([P, P], f32)
            nc.sync.dma_start(out=ident[:, :], in_=Idn[:, :])
            wt_sb = constp.tile([P, F], bf16)
            nc.sync.dma_start(out=wt_sb[:, :], in_=Wt[:, :])
            bb_sb = constp.tile([P, F], f32)
            nc.sync.dma_start(out=bb_sb[:, :], in_=Bb[:, :])
            sel_sb = constp.tile([JT, IT], f32)
            nc.sync.dma_start(out=sel_sb[:, :], in_=Sel[:, :])
            ones = constp.tile([P, P], bf16)
            nc.vector.memset(ones[:, :], 1.0)

            # ---- persistent big buffers ----
            At = bigp.tile([P, JT * SR], bf16)   # transposed strip, bf16
            Usb = bigp.tile([P, N], bf16)        # U tiles, then Y = dinv*U
            xt_sb = bigp.tile([P, N], bf16)

            # ---- stream A strip (the only big HBM read); X^T last ----
            for jt in range(JT):
                eng = nc.sync if jt % 2 == 0 else nc.scalar
                eng.dma_start(
                    out=At[:, jt * SR:(jt + 1) * SR],
                    in_=At_d[jt * P:(jt + 1) * P, :],
                )
            nc.sync.dma_start(out=xt_sb[:, :], in_=Xt[:, :])

            # ---- degrees: deg[i] = sum_k at[k, i], all-ones matmuls ----
            # ones stationary is [128,128] so the PSUM drain spreads across
            # all partitions (M=1 serializes the drain and halves PE rate);
            # every output row holds the same column sums.
            degPs = [pdeg.tile([P, HC], f32, name=f"degP{h}", bufs=1)
                     for h in range(2)]
            for jt in range(JT):
                for h in range(2):
                    nc.tensor.matmul(
                        degPs[h][0:64, :],
                        ones[:, 0:64],
                        At[:, jt * SR + h * HC: jt * SR + (h + 1) * HC],
                        start=(jt == 0), stop=(jt == JT - 1),
                    )
            degS = smallp.tile([1, SR], f32)
            nc.scalar.copy(degS[0:1, 0:HC], degPs[0][0:1, :])
            nc.scalar.copy(degS[0:1, HC:SR], degPs[1][0:1, :])
            nc.sync.dma_start(out=degL[:, :], in_=degS[:, :])

            # ---- AllGather local degrees -> full degree ----
            nc.gpsimd.collective_compute(
                "AllGather", mybir.AluOpType.bypass,
                replica_groups=[list(range(NCORES))],
                ins=[degL[:, :]], outs=[degA[:, :]],
            )
            deg_sb = smallp.tile([JT, P], f32)
            nc.sync.dma_start(out=deg_sb[:, :], in_=degA[:, :])

            # ---- U = X @ W^T (64 small matmuls; fill the CC window) ----
            for jt in range(JT):
                up = pu.tile([P, F], f32)
                nc.tensor.matmul(
                    up[:, :], xt_sb[:, jt * P:(jt + 1) * P], wt_sb[:, :],
                    start=True, stop=True,
                )
                nc.vector.tensor_copy(Usb[:, jt * F:(jt + 1) * F], up[:, :])

            # ---- dinv = deg^-1/2: Sqrt LUT + vector reciprocal (the
            # bass-sanctioned pair; LUT error ~1e-5 is invisible next to
            # the bf16-dominated 3.5e-3 budget, so no Newton polish) ----
            dinvG = smallp.tile([JT, P], f32)
            sqG = smallp.tile([JT, P], f32)
            nc.scalar.activation(sqG[:, :], deg_sb[:, :], AF.Sqrt)
            nc.vector.reciprocal(dinvG[:, :], sqG[:, :])

            # dinvT [128, 64]: pad to [128,128], PE transpose
            dpad = smallp.tile([P, P], f32)
            nc.vector.memset(dpad[:, :], 0.0)
            nc.vector.tensor_copy(dpad[0:JT, :], dinvG[:, :])
            dps = ptr.tile([P, P], f32, tag="tr")
            nc.tensor.transpose(dps[:, :], dpad[:, :], ident[:, :])
            dinvT = smallp.tile([P, JT], f32)
            nc.vector.tensor_copy(dinvT[:, :], dps[:, 0:JT])

            # dinvL [128, 8]: select local rows then transpose
            lps = ptr.tile([P, P], f32, tag="tr")
            nc.tensor.matmul(lps[0:IT, :], sel_sb[:, :], dinvG[:, :],
                             start=True, stop=True)
            lsel = smallp.tile([IT, P], f32)
            nc.vector.tensor_copy(lsel[:, :], lps[0:IT, :])
            lts = ptr.tile([P, P], f32, tag="tr")
            nc.tensor.transpose(lts[:, 0:IT], lsel[:, :], ident[0:IT, 0:IT])
            dinvL = smallp.tile([P, IT], f32)
            nc.vector.tensor_copy(dinvL[:, :], lts[:, 0:IT])

            # ---- Y = dinv * U (bf16, in place) ----
            for jt in range(JT):
                nc.vector.tensor_scalar_mul(
                    Usb[:, jt * F:(jt + 1) * F], Usb[:, jt * F:(jt + 1) * F],
                    dinvT[:, jt:jt + 1],
                )

            # ---- phase 2: Z^T[f, i] = sum_k Y[k, f] at[k, i] ----
            zts = [pzt.tile([P, HC], f32, name=f"zt{h}", bufs=1) for h in range(2)]

            def mm_h(h, jt):
                nc.tensor.matmul(
                    zts[h][:, :],
                    Usb[:, jt * F:(jt + 1) * F],
                    At[:, jt * SR + h * HC: jt * SR + (h + 1) * HC],
                    start=(jt == 0), stop=(jt == JT - 1),
                )

            # epilogue: transpose back, row scale, bias, store
            def epi(h):
                ztS = outp.tile([P, HC], f32)
                nc.vector.tensor_copy(ztS[:, :], zts[h][:, :])
                for q in range(4):
                    it = h * 4 + q
                    tp = ptr.tile([P, P], f32, tag="tr")
                    nc.tensor.transpose(tp[:, :], ztS[:, q * P:(q + 1) * P],
                                        ident[:, :])
                    o = outp.tile([P, F], f32)
                    nc.vector.tensor_scalar_mul(o[:, :], tp[:, :],
                                                dinvL[:, it:it + 1])
                    nc.vector.tensor_add(o[:, :], o[:, :], bb_sb[:, :])
                    nc.sync.dma_start(out=out[it * P:(it + 1) * P, :],
                                      in_=o[:, :])

            # half-0 accumulation; its epilogue overlaps half 1's stream
            for jt in range(JT):
                mm_h(0, jt)
            for jt in range(8):
                mm_h(1, jt)
            epi(0)
            for jt in range(8, JT):
                mm_h(1, jt)
            epi(1)

    return nc


_NO_SPLIT_TYPES = ("InstEventSemaphore", "InstSemaphore", "InstTrigger")


def _split_drain_waits(nc, max_waits=1):
    """This walrus build only encodes one sem-wait per instruction; hoist
    extras onto preceding same-engine NOPs (monotonic sems => equivalent)."""
    import concourse.mybir as mybir
    for fn in nc.m.functions:
        for blk in fn.blocks:
            newlist = []
            for ins in blk.instructions:
                si = getattr(ins, "sync_info", None)
                tname = type(ins).__name__
                if si is not None and si.on_wait and len(si.on_wait) > max_waits \
                        and not any(tname.startswith(t) for t in _NO_SPLIT_TYPES):
                    waits = list(si.on_wait)
                    for j, w in enumerate(waits[max_waits:]):
                        newlist.append(mybir.InstNoOp(
                            name=f"{ins.name}-w{j}", engine=ins.engine,
                            ins=[], outs=[],
                            sync_info=mybir.SyncInfo(on_wait=[w], on_update=[]),
                        ))
                    si.on_wait = waits[:max_waits]
                newlist.append(ins)
            blk.instructions[:] = newlist


def _get_nc():
    if "nc" not in _CACHE:
        nc = _build_nc()
        _split_drain_waits(nc)
        _CACHE["nc"] = nc
    return _CACHE["nc"]


def _make_in_maps(X, A, W, b):
    bf16 = ml_dtypes.bfloat16
    X = np.ascontiguousarray(np.asarray(X, dtype=np.float32))
    A = np.ascontiguousarray(np.asarray(A, dtype=np.float32))
    W = np.ascontiguousarray(np.asarray(W, dtype=np.float32))
    b = np.ascontiguousarray(np.asarray(b, dtype=np.float32))
    Xt_bf = np.ascontiguousarray(X.T).astype(bf16)
    Wt_bf = np.ascontiguousarray(W.T).astype(bf16)
    Bb = np.ascontiguousarray(np.tile(b[None, :], (P, 1)))
    Idn = np.eye(P, dtype=np.float32)
    idx = np.arange(SR)
    in_maps = []
    for c in range(NCORES):
        at = A[c * SR:(c + 1) * SR, :].T.astype(bf16)  # [N, SR], contiguous
        at[c * SR + idx, idx] += np.float32(1.0)       # self loops (A + I)
        sel = np.zeros((JT, IT), dtype=np.float32)
        sel[c * IT + np.arange(IT), np.arange(IT)] = 1.0
        in_maps.append({
            "at_hat": at,
            "xt_bf": Xt_bf,
            "wt_bf": Wt_bf,
            "b_bc": Bb,
            "ident": Idn,
            "sel": sel,
        })
    return in_maps


def _install_ntff_hook():
    """This image's antenv lacks axon_hooks; synthesize it so trace=True
    can reach the terminal's NTFF capture via the libaxon ctypes hook."""
    import sys
    import types
    if "antenv.axon_hooks" in sys.modules:
        return
    try:
        from trn_agent_boot.trn_boot import _ntff_profile_via_ctypes
        hook = _ntff_profile_via_ctypes("/opt/axon/libaxon_pjrt.so")
    except Exception:
        hook = None
    mod = types.ModuleType("antenv.axon_hooks")
    mod._hook = hook
    mod.get_axon_ntff_profile_hook = lambda: mod._hook
    def _set(h):
        mod._hook = h
    mod.set_axon_ntff_profile_hook = _set
    sys.modules["antenv.axon_hooks"] = mod
    import antenv
    antenv.axon_hooks = mod
    # the artifact upload needs a bucket this sandbox doesn't have
    import concourse.bass_utils as bu
    bu.upload_artifacts = lambda tmpdir: f"local:{tmpdir}"


def run(X, A, W, b, trace=False, **trace_kwargs):
    """Run on hardware; returns (output, BassKernelResults)."""
    from concourse.bass_utils import run_bass_kernel_spmd
    if trace:
        _install_ntff_hook()
    nc = _get_nc()
    in_maps = _make_in_maps(X, A, W, b)
    res = run_bass_kernel_spmd(nc, in_maps, list(range(NCORES)),
                               trace=trace, **trace_kwargs)
    outs = [np.asarray(res.results[c]["out"], dtype=np.float32)
            for c in range(NCORES)]
    return np.concatenate(outs, axis=0), res


def kernel(X, A, W, b):
    out, _ = run(X, A, W, b, trace=False)
    return out



# revision 3
# speedup vs baseline: 1.0814x; 1.0814x over previous
"""GCN layer kernel for 8 trn2 NeuronCores (SPMD, single launch).

Math:  out = D^-1/2 (A+I) D^-1/2 X W^T + b
Identity: the dense layer commutes with the diagonal scalings:
    out = D^-1/2 (A+I) D^-1/2 (X W^T) + b
so U = X@W^T (tiny) is computed first, then one big matmul A_hat @ (dinv*U).

Distribution: row-shard A_hat = A+I across 8 cores (strip = 1024 rows).
The host supplies each core's strip TRANSPOSED, CENTERED (-0.5) and cast
to fp8e4m3 in a k-tile-packed layout (at[p, jt*1024+i] = A_hat[row i,
jt*128+p] - 0.5), which
  * quarters HBM traffic vs fp32 (8.4MB/core, ~24us at 358GB/s roofline),
  * makes every stream DMA a 512KB contiguous-per-partition transfer, and
  * puts the contraction dim k on partitions, so no on-device transposes.
Centering halves the fp8 quantization error (values in [-.5,.5) instead of
[0,1)); the rank-1 0.5*ones term is restored exactly via a colsum(Y)
correction in the epilogue, and degrees via a +N/2 shift.

Per core:
  warmup:  a dummy 32B AllGather fires at t~0 so the ncfw rendezvous /
      first-collective barrier is absorbed under the A stream.
  phase 1 (overlapped): stream the fp8 strip in 16 512KB DMAs on the two
      HWDGE queues; degrees deg[i] = sum_k at[k, i] via ones-matmuls
      (PSUM accum, bf16 x fp8); local dinv = (deg+4096)^-1/2 computed
      BEFORE the collective so the gather ships final dinv values.
  AllGather (the only data collective): 1024 local dinv -> full 8192.
  X^T loads after A (off the critical HBM window); U = X@W^T plus filler
      matmuls keep the PE clock-gate warm across the collective.
  phase 2: Y = dinv*U (bf16, in place, per k-tile, pipelined);
      Z^T[f, i] = sum_k Y[k, f] at[k, i] accumulated over 64 k-tiles
      (Y tile stationary, 512-wide fp8 streams); colsum(Y) accumulated on
      DVE for the centering correction;
      epilogue: PE-transpose Z^T tiles, out = dinvL*Z + dinvL*0.5*s + b.

A is read from HBM exactly once, in fp8.
"""

import numpy as np
import ml_dtypes

N = 8192          # nodes
F = 128           # in/out feature dim
NCORES = 8
SR = N // NCORES  # strip rows per core = 1024
P = 128           # partitions / tile edge
IT = SR // P      # 8 local row tiles
JT = N // P       # 64 contraction tiles
HC = 512          # phase-2 / degree stream chunk (one PSUM bank of fp32)
NCHUNK = 16       # A-stream DMA chunks (512KB each)
CW = N * SR // NCHUNK // P  # 4096 fp8 elems per partition per chunk
MEAN = 0.5        # subtracted from A_hat on host, restored on device
WARMUP_CC = True  # dummy collective at t~0 to absorb the cc rendezvous
NDUM = 24         # PE warm-keeper matmuls across the collective window

_CACHE = {}


def _build_nc():
    import concourse.mybir as mybir
    from concourse import bass
    from concourse.tile import TileContext

    f32 = mybir.dt.float32
    bf16 = mybir.dt.bfloat16
    f8 = mybir.dt.float8e4
    AF = mybir.ActivationFunctionType

    nc = bass.Bass(num_devices=NCORES)

    At_d = nc.declare_dram_parameter("at_f8", [P, N * SR // P], f8, False)
    Xt_d = nc.declare_dram_parameter("xt_bf", [P, N], bf16, False)    # X^T
    Cpk_d = nc.declare_dram_parameter("cpack", [P, 392], f32, False)
    Wpk_d = nc.declare_dram_parameter("wpack", [P, 192], bf16, False)
    out = nc.declare_dram_parameter("out", [SR, F], f32, True)

    degL = nc.dram_tensor("deg_local", [IT, P], f32)
    degA = nc.dram_tensor("deg_all", [JT, P], f32, addr_space="Shared")
    if WARMUP_CC:
        dumL = nc.dram_tensor("dum_local", [1, 8], f32)
        dumA = nc.dram_tensor("dum_all", [NCORES, 8], f32,
                              addr_space="Shared")

    rg = [list(range(NCORES))]

    with TileContext(nc) as tc:
        with tc.tile_pool(name="const", bufs=1) as constp, \
             tc.tile_pool(name="big", bufs=1) as bigp, \
             tc.tile_pool(name="small", bufs=1) as smallp, \
             tc.tile_pool(name="outs", bufs=3) as outp, \
             tc.tile_pool(name="pdeg", bufs=1, space="PSUM") as pdeg, \
             tc.tile_pool(name="pu", bufs=2, space="PSUM") as pu, \
             tc.tile_pool(name="pzt", bufs=2, space="PSUM") as pzt, \
             tc.tile_pool(name="ptr", bufs=2, space="PSUM") as ptr:

            # ---- warmup collective: fires immediately, runs under the
            # stream, so the first-collective rendezvous + ncfw wake cost
            # is not paid by the real degree gather ----
            if WARMUP_CC:
                dsrc = smallp.tile([1, 8], f32)
                nc.vector.memset(dsrc[:, :], 0.0)
                nc.gpsimd.dma_start(out=dumL[:, :], in_=dsrc[:, :])
                nc.gpsimd.collective_compute(
                    "AllGather", mybir.AluOpType.bypass,
                    replica_groups=rg,
                    ins=[dumL[:, :]], outs=[dumA[:, :]],
                )

            # ---- packed constants (2 small DMAs ahead of the A stream) --
            cpk = constp.tile([P, 392], f32)
            nc.sync.dma_start(out=cpk[:, :], in_=Cpk_d[:, :])
            wpk = constp.tile([P, 192], bf16)
            nc.sync.dma_start(out=wpk[:, :], in_=Wpk_d[:, :])
            ident = cpk[:, 0:P]            # I_128 (fp32)
            bb_sb = cpk[:, P:2 * P]        # bias broadcast [128, F]
            sel8 = cpk[0:JT, 2 * P:2 * P + IT]   # local-rows one-hot [64,8]
            onesF = cpk[:, 264:392]        # all-ones fp32 block
            wt_sb = wpk[:, 0:F]            # W^T bf16 [128, 128]
            onesB = wpk[:, F:F + 64]       # all-ones bf16 [128, 64]

            # ---- persistent big buffers ----
            At = bigp.tile([P, N * SR // P], f8)   # packed strip, fp8
            Usb = bigp.tile([P, N], bf16)          # U tiles, then Y
            xt_sb = bigp.tile([P, N], bf16)

            # ---- stream A strip: 16 x 512KB on the two HWDGE queues ----
            for ch in range(NCHUNK):
                eng = nc.sync if ch % 2 == 0 else nc.scalar
                eng.dma_start(
                    out=At[:, ch * CW:(ch + 1) * CW],
                    in_=At_d[:, ch * CW:(ch + 1) * CW],
                )
            # X^T after A on the sync queue (4 chunks so U can start early)
            for xc in range(4):
                nc.sync.dma_start(
                    out=xt_sb[:, xc * (N // 4):(xc + 1) * (N // 4)],
                    in_=Xt_d[:, xc * (N // 4):(xc + 1) * (N // 4)],
                )

            # ---- degrees: deg[i] = sum_k at[k, i], all-ones matmuls ----
            degPs = [pdeg.tile([64, HC], f32, name=f"degP{h}", bufs=1)
                     for h in range(2)]
            for jt in range(JT):
                for h in range(2):
                    nc.tensor.matmul(
                        degPs[h][:, :],
                        onesB[:, :],
                        At[:, jt * SR + h * HC: jt * SR + (h + 1) * HC],
                        start=(jt == 0), stop=(jt == JT - 1),
                    )

            # ---- local dinv before the gather (shorter post-cc tail) ----
            degS = smallp.tile([1, SR], f32)
            nc.scalar.copy(degS[0:1, 0:HC], degPs[0][0:1, :])
            nc.scalar.copy(degS[0:1, HC:SR], degPs[1][0:1, :])
            # deg = colsum(C) + N*MEAN;  dinv = 1/sqrt(deg)
            nc.vector.tensor_scalar_add(degS[:, :], degS[:, :],
                                        float(N * MEAN))
            sqS = smallp.tile([1, SR], f32)
            nc.scalar.activation(sqS[:, :], degS[:, :], AF.Sqrt)
            dinvS = smallp.tile([1, SR], f32)
            nc.vector.reciprocal(dinvS[:, :], sqS[:, :])
            nc.gpsimd.dma_start(out=degL[:, :], in_=dinvS[:, :])

            # ---- AllGather local dinv -> full dinv ----
            nc.gpsimd.collective_compute(
                "AllGather", mybir.AluOpType.bypass,
                replica_groups=rg,
                ins=[degL[:, :]], outs=[degA[:, :]],
            )

            # ---- PE warm work across the collective window ----
            # filler matmuls into the (drained) degree banks
            for d in range(NDUM // 2):
                nc.tensor.matmul(degPs[d % 2][:, :], onesB[:, :],
                                 At[:, (d % IT) * HC:(d % IT + 1) * HC],
                                 start=True, stop=True)
            # U = X @ W^T (64 small matmuls)
            for jt in range(JT):
                up = pu.tile([P, F], f32)
                nc.tensor.matmul(
                    up[:, :], xt_sb[:, jt * P:(jt + 1) * P], wt_sb[:, :],
                    start=True, stop=True,
                )
                nc.vector.tensor_copy(Usb[:, jt * F:(jt + 1) * F], up[:, :])
            for d in range(NDUM - NDUM // 2):
                nc.tensor.matmul(degPs[d % 2][:, :], onesB[:, :],
                                 At[:, (d % IT) * HC:(d % IT + 1) * HC],
                                 start=True, stop=True)

            # ---- post-gather: dinvT [128, 64] and dinvL [128, 8] ----
            dinvG = smallp.tile([JT, P], f32)
            nc.sync.dma_start(out=dinvG[:, :], in_=degA[:, :])
            tp1 = ptr.tile([P, JT], f32, tag="tr")
            nc.tensor.matmul(tp1[:, :], dinvG[:, :], ident[0:JT, 0:JT],
                             start=True, stop=True)
            dinvT = smallp.tile([P, JT], f32)
            nc.vector.tensor_copy(dinvT[:, :], tp1[:, :])
            tp2 = ptr.tile([P, IT], f32, tag="tr")
            nc.tensor.matmul(tp2[:, :], dinvG[:, :], sel8[:, :],
                             start=True, stop=True)
            dinvL = smallp.tile([P, IT], f32)
            nc.vector.tensor_copy(dinvL[:, :], tp2[:, :])

            # ---- phase 2: Y = dinv*U; Z^T = sum_k Y[k,f] at[k,i];
            #      acc = colsum(Y) for the centering correction ----
            zts = [pzt.tile([P, HC], f32, name=f"zt{h}", bufs=1)
                   for h in range(2)]
            acc = smallp.tile([P, F], f32)
            nc.vector.memset(acc[:, :], 0.0)
            for jt in range(JT):
                ut = Usb[:, jt * F:(jt + 1) * F]
                nc.vector.tensor_scalar_mul(ut, ut, dinvT[:, jt:jt + 1])
                for h in range(2):
                    nc.tensor.matmul(
                        zts[h][:, :], ut,
                        At[:, jt * SR + h * HC: jt * SR + (h + 1) * HC],
                        start=(jt == 0), stop=(jt == JT - 1),
                    )
                nc.vector.tensor_add(acc[:, :], acc[:, :], ut)

            # s = MEAN * colsum(Y) broadcast to all partitions
            sps = ptr.tile([1, F], f32, tag="tr")
            nc.tensor.matmul(sps[0:1, :], onesF[:, 0:1], acc[:, :],
                             start=True, stop=True)
            s_sb = smallp.tile([1, F], f32)
            nc.scalar.mul(s_sb[:, :], sps[0:1, :], float(MEAN))
            sbps = ptr.tile([P, F], f32, tag="tr")
            nc.tensor.matmul(sbps[:, :], onesF[0:1, :], s_sb[:, :],
                             start=True, stop=True)
            s_bc = smallp.tile([P, F], f32)
            nc.vector.tensor_copy(s_bc[:, :], sbps[:, :])
            # bcb[it] = dinvL[:, it] * s_bc + b  (per-row correction + bias)
            bcb = smallp.tile([P, IT * F], f32)
            for it in range(IT):
                nc.vector.tensor_scalar_mul(
                    bcb[:, it * F:(it + 1) * F], s_bc[:, :],
                    dinvL[:, it:it + 1])
                nc.vector.tensor_add(
                    bcb[:, it * F:(it + 1) * F],
                    bcb[:, it * F:(it + 1) * F], bb_sb[:, :])

            # ---- epilogue: transpose back, row scale, correction ----
            def epi(h):
                ztS = outp.tile([P, HC], f32)
                nc.vector.tensor_copy(ztS[:, :], zts[h][:, :])
                for q in range(4):
                    it = h * 4 + q
                    tp = ptr.tile([P, P], f32, tag="tr")
                    nc.tensor.transpose(tp[:, :], ztS[:, q * P:(q + 1) * P],
                                        ident[:, :])
                    o = outp.tile([P, F], f32)
                    nc.vector.tensor_scalar_mul(o[:, :], tp[:, :],
                                                dinvL[:, it:it + 1])
                    nc.vector.tensor_add(o[:, :], o[:, :],
                                         bcb[:, it * F:(it + 1) * F])
                    nc.scalar.dma_start(out=out[it * P:(it + 1) * P, :],
                                        in_=o[:, :])

            epi(0)
            epi(1)

    return nc


_NO_SPLIT_TYPES = ("InstEventSemaphore", "InstSemaphore", "InstTrigger")


def _split_drain_waits(nc, max_waits=1):
    """This walrus build only encodes one sem-wait per instruction; hoist
    extras onto preceding same-engine NOPs (monotonic sems => equivalent)."""
    import concourse.mybir as mybir
    for fn in nc.m.functions:
        for blk in fn.blocks:
            newlist = []
            for ins in blk.instructions:
                si = getattr(ins, "sync_info", None)
                tname = type(ins).__name__
                if si is not None and si.on_wait and len(si.on_wait) > max_waits \
                        and not any(tname.startswith(t) for t in _NO_SPLIT_TYPES):
                    waits = list(si.on_wait)
                    for j, w in enumerate(waits[max_waits:]):
                        newlist.append(mybir.InstNoOp(
                            name=f"{ins.name}-w{j}", engine=ins.engine,
                            ins=[], outs=[],
                            sync_info=mybir.SyncInfo(on_wait=[w], on_update=[]),
                        ))
                    si.on_wait = waits[:max_waits]
                newlist.append(ins)
            blk.instructions[:] = newlist


def _get_nc():
    if "nc" not in _CACHE:
        nc = _build_nc()
        _split_drain_waits(nc)
        _CACHE["nc"] = nc
    return _CACHE["nc"]


def _make_in_maps(X, A, W, b):
    bf16 = ml_dtypes.bfloat16
    f8 = ml_dtypes.float8_e4m3
    X = np.ascontiguousarray(np.asarray(X, dtype=np.float32))
    A = np.ascontiguousarray(np.asarray(A, dtype=np.float32))
    W = np.ascontiguousarray(np.asarray(W, dtype=np.float32))
    b = np.ascontiguousarray(np.asarray(b, dtype=np.float32))
    Xt_bf = np.ascontiguousarray(X.T).astype(bf16)

    cpack = np.zeros((P, 392), dtype=np.float32)
    cpack[:, 0:P] = np.eye(P, dtype=np.float32)
    cpack[:, P:2 * P] = np.tile(b[None, :], (P, 1))
    cpack[:, 264:392] = 1.0
    wpack = np.zeros((P, 192), dtype=np.float32)
    wpack[:, 0:F] = W.T
    wpack[:, F:F + 64] = 1.0
    wpack = wpack.astype(bf16)

    idx = np.arange(SR)
    in_maps = []
    for c in range(NCORES):
        at = A[c * SR:(c + 1) * SR, :].T.astype(np.float32)  # [N, SR]
        at[c * SR + idx, idx] += np.float32(1.0)             # self loops
        at -= np.float32(MEAN)                               # centering
        at8 = at.astype(f8)
        # pack k-tiles: pk[p, jt*SR + i] = at[jt*P + p, i]
        pk = np.ascontiguousarray(
            at8.reshape(JT, P, SR).transpose(1, 0, 2).reshape(P, N * SR // P))
        cp = cpack.copy()
        cp[0:JT, 2 * P:2 * P + IT] = 0.0
        cp[c * IT + np.arange(IT), 2 * P + np.arange(IT)] = 1.0
        in_maps.append({
            "at_f8": pk,
            "xt_bf": Xt_bf,
            "cpack": cp,
            "wpack": wpack,
        })
    return in_maps


def _install_ntff_hook():
    """This image's antenv lacks axon_hooks; synthesize it so trace=True
    can reach the terminal's NTFF capture via the libaxon ctypes hook."""
    import sys
    import types
    if "antenv.axon_hooks" in sys.modules:
        return
    try:
        from trn_agent_boot.trn_boot import _ntff_profile_via_ctypes
        hook = _ntff_profile_via_ctypes("/opt/axon/libaxon_pjrt.so")
    except Exception:
        hook = None
    mod = types.ModuleType("antenv.axon_hooks")
    mod._hook = hook
    mod.get_axon_ntff_profile_hook = lambda: mod._hook
    def _set(h):
        mod._hook = h
    mod.set_axon_ntff_profile_hook = _set
    sys.modules["antenv.axon_hooks"] = mod
    import antenv
    antenv.axon_hooks = mod
    # the artifact upload needs a bucket this sandbox doesn't have
    import concourse.bass_utils as bu
    bu.upload_artifacts = lambda tmpdir: f"local:{tmpdir}"


def run(X, A, W, b, trace=False, **trace_kwargs):
    """Run on hardware; returns (output, BassKernelResults)."""
    from concourse.bass_utils import run_bass_kernel_spmd
    if trace:
        _install_ntff_hook()
    nc = _get_nc()
    in_maps = _make_in_maps(X, A, W, b)
    res = run_bass_kernel_spmd(nc, in_maps, list(range(NCORES)),
                               trace=trace, **trace_kwargs)
    outs = [np.asarray(res.results[c]["out"], dtype=np.float32)
            for c in range(NCORES)]
    return np.concatenate(outs, axis=0), res


def kernel(X, A, W, b):
    out, _ = run(X, A, W, b, trace=False)
    return out
